# revision 24
# baseline (speedup 1.0000x reference)
"""Trainium2 Bass kernel: per-head (head_dim=128) Walsh-Hadamard transform.

Full input  : value [16384, 4096] f32  (= [tokens, 32 heads * 128])
Full output : same shape; out[t, h*128:(h+1)*128] = (H_128 @ v) / sqrt(128)

Strategy (pure data parallel over tokens, 8 cores, 2048 tokens each):
  - All input DMAs on the single SP HWDGE ring; all steady-state output DMAs
    on the SWDGE (gpsimd) ring.  SDMA round-robins between the two queues at
    packet granularity, so the in/out HBM split stays ~50/50 and outputs
    never build a large backlog for the tail to drain.
  - Per 128x128 head block B: PE transpose (fp32, 2 cyc/row) B -> B^T in
    PSUM; ScalarE copies PSUM -> SBUF casting to bf16 (ACT has its own SBUF
    ports — a 16-bit DVE op would enter 2-port perf mode and starve SWDGE
    descriptor generation); PE matmul Z = (B^T).T @ H with bf16 operands
    (1 cyc/row vs 4 for fp32; H entries +-1 are exact in bf16) -> fp32 PSUM;
    DVE tensor_scalar_mul moves Z PSUM -> SBUF with the 1/sqrt(128) scale
    (fp32, 1-port mode).
  - Graduated chunk schedule: narrow chunks at the start (fast pipeline
    ramp), 1 MiB chunks steady-state, per-group drains of the final tile on
    the by-then-idle SP ring (short tail).
Measured: 192789 ns vs 202456 ns staged baseline; rel err 1.66e-3.
"""

import math

import ml_dtypes
import numpy as np

import concourse.bass as bass  # noqa: F401  (AP helpers)
import concourse.mybir as mybir
import concourse.tile as tile
from concourse import bacc
from concourse.bass_utils import run_bass_kernel_spmd

HEAD_DIM = 128
N_CORES = 8
TOKENS = 16384
HIDDEN = 4096
P = 128  # partitions / tile token rows


def _hadamard(n: int) -> np.ndarray:
    h = np.array([[1.0]], dtype=np.float64)
    while h.shape[0] < n:
        h = np.block([[h, h], [h, -h]])
    return h


def build_nc(tok_per_core: int = TOKENS // N_CORES, hidden: int = HIDDEN,
             group_heads: int = 4, chunk_cols: int = 2048,
             xin_bufs: int = 8, out_bufs: int = 6, xt_bufs: int = 4,
             pt_bufs: int = 4, pz_bufs: int = 4):
    """Build the per-core Bass program.

    group_heads 128-wide head blocks are batched into one PSUM bank
    ([128, group_heads*128] f32).  chunk_cols is the DMA chunk width: each
    in/out DMA moves [128, chunk_cols] f32 so the pipeline starts early and
    drains late with ~chunk-sized latency instead of full-row latency.
    """
    gw = group_heads * HEAD_DIM  # group width in columns
    assert tok_per_core % P == 0 and hidden % gw == 0
    assert chunk_cols % gw == 0 and hidden % chunk_cols == 0
    n_tiles = tok_per_core // P
    n_chunks = hidden // chunk_cols
    groups_per_chunk = chunk_cols // gw
    scale = float(np.float32(1.0 / math.sqrt(HEAD_DIM)))

    nc = bacc.Bacc("TRN2", target_bir_lowering=False)
    x = nc.dram_tensor("x", [tok_per_core, hidden], mybir.dt.float32,
                       kind="ExternalInput")
    out = nc.dram_tensor("out", [tok_per_core, hidden], mybir.dt.float32,
                         kind="ExternalOutput")
    hm = nc.inline_tensor(
        _hadamard(HEAD_DIM).astype(ml_dtypes.bfloat16), "hm")
    ident = nc.inline_tensor(np.eye(HEAD_DIM, dtype=np.float32), "ident")

    with tile.TileContext(nc) as tc:
        with (
            tc.tile_pool(name="consts", bufs=1) as cpool,
            tc.tile_pool(name="xin", bufs=xin_bufs) as xpool,
            tc.tile_pool(name="xtb", bufs=xt_bufs) as xtpool,
            tc.tile_pool(name="outb", bufs=out_bufs) as opool,
            tc.tile_pool(name="pt", bufs=pt_bufs, space="PSUM") as ptpool,
            tc.tile_pool(name="pz", bufs=pz_bufs, space="PSUM") as pzpool,
        ):
            hm_sb = cpool.tile([HEAD_DIM, HEAD_DIM], mybir.dt.bfloat16)
            nc.gpsimd.dma_start(hm_sb[:], hm[:])
            id_sb = cpool.tile([HEAD_DIM, HEAD_DIM], mybir.dt.float32)
            nc.gpsimd.dma_start(id_sb[:], ident[:])

            # Flat chunk schedule: graduated chunk widths — small at the very
            # start (so the first transpose begins after a tiny DMA instead
            # of 1MiB fair-shared against 5 other prefetches), ramping up to
            # chunk_cols, small again at the very end (short output drain).
            # Last-tile outputs go via the HWDGE rings so the SWDGE ring
            # drains early, off the critical path.
            sched = []  # (row, c0, width)
            for i in range(n_tiles):
                if i == 0:
                    w = gw
                    for ch in range(hidden // w):
                        sched.append((i, ch * w, w))
                elif i == 1:
                    w = max(gw, chunk_cols // 2)
                    for ch in range(hidden // w):
                        sched.append((i, ch * w, w))
                else:
                    for ch in range(n_chunks):
                        sched.append((i, ch * chunk_cols, chunk_cols))

            for k, (i, c0, w) in enumerate(sched):
                x_tile = xpool.tile([P, chunk_cols], mybir.dt.float32)
                # ALL inputs on the single SP HWDGE ring: 50/50 in/out
                # packet round-robin against the SWDGE output queue
                nc.sync.dma_start(
                    x_tile[:, :w], x[i * P:(i + 1) * P, c0:c0 + w])
                o_tile = opool.tile([P, chunk_cols], mybir.dt.float32)
                egw = gw
                for g in range(w // egw):
                    pt = ptpool.tile([P, egw], mybir.dt.float32)
                    for j in range(egw // HEAD_DIM):
                        c = g * egw + j * HEAD_DIM
                        nc.tensor.transpose(
                            pt[:, j * HEAD_DIM:(j + 1) * HEAD_DIM],
                            x_tile[:, c:c + HEAD_DIM],
                            id_sb[:],
                        )
                    # PSUM fp32 -> SBUF bf16 cast on ACT (own SBUF ports —
                    # a 16-bit DVE op here would enter 2-port perf mode and
                    # starve GpSimd's SWDGE descriptor generation); bf16
                    # operands let matmul #2 stream 1 cyc/row vs 4 for fp32
                    # at ~0.2% rel err
                    xt_sb = xtpool.tile([P, egw], mybir.dt.bfloat16)
                    nc.scalar.copy(xt_sb[:], pt[:])
                    pz = pzpool.tile([P, egw], mybir.dt.float32)
                    for j in range(egw // HEAD_DIM):
                        nc.tensor.matmul(
                            pz[:, j * HEAD_DIM:(j + 1) * HEAD_DIM],
                            xt_sb[:, j * HEAD_DIM:(j + 1) * HEAD_DIM],
                            hm_sb[:],
                        )
                    # fp32 scale-move on DVE stays in 1-port mode (same as
                    # the baseline's fp32 copy, which didn't perturb SWDGE)
                    nc.vector.tensor_scalar_mul(
                        o_tile[:, g * egw:(g + 1) * egw], pz[:], scale)
                    if i == n_tiles - 1:
                        nc.sync.dma_start(
                            out[i * P:(i + 1) * P,
                                c0 + g * egw:c0 + (g + 1) * egw],
                            o_tile[:, g * egw:(g + 1) * egw])
                if i < n_tiles - 1:
                    # outputs via SWDGE (gpsimd) — a third DGE path so output
                    # readiness never head-of-line-blocks the input rings
                    # (outputs on the HWDGE rings measured +28us: an output
                    # dma_start waiting on its mul stalls every later input
                    # dma queued behind it on that engine's FIFO)
                    nc.gpsimd.dma_start(
                        out[i * P:(i + 1) * P, c0:c0 + w], o_tile[:, :w])
    nc.finalize()
    return nc


_NC_CACHE = {}


def _get_nc(tok_per_core: int, hidden: int):
    key = (tok_per_core, hidden)
    if key not in _NC_CACHE:
        _NC_CACHE[key] = build_nc(tok_per_core, hidden)
    return _NC_CACHE[key]


def kernel(value, **_unused) -> np.ndarray:
    value = np.ascontiguousarray(np.asarray(value), dtype=np.float32)
    tokens, hidden = value.shape
    assert tokens % N_CORES == 0
    tok_per_core = tokens // N_CORES
    nc = _get_nc(tok_per_core, hidden)
    shards = np.split(value, N_CORES, axis=0)
    in_maps = [{"x": s} for s in shards]
    res = run_bass_kernel_spmd(nc, in_maps, core_ids=list(range(N_CORES)))
    return np.concatenate([r["out"] for r in res.results], axis=0)

